# revision 4
# baseline (speedup 1.0000x reference)
"""Trainium2 Bass kernel for a rate-1/2, constraint-length-3 feedforward
convolutional encoder (generator polynomials "101" and "111", MSB-first).

The trellis scan in the reference collapses to elementwise XORs of shifted
input bits (zero initial state):

    out0[t] = u[t] ^ u[t-2]            (poly "101")
    out1[t] = u[t] ^ u[t-1] ^ u[t-2]   (poly "111")

with the codeword interleaved time-major: y[:, 2t] = out0[t], y[:, 2t+1] = out1[t].

XOR on {0,1} f32 values is computed bitwise on uint32 views (1.0f =
0x3F800000, 0.0f = 0x0), one DVE op per output stream.

DMA layout: the kernel is bound by the ~430 GB/s/core SDMA-engine
aggregate. Inputs are issued upfront, alternating across both HWDGE rings
(SP + ACT) so reads ramp immediately; outputs follow on the same two
rings once each group's two XORs finish. GpSimd issues no DMAs at all --
a continuously-busy DVE locks GpSimd out of the shared SBUF port pair,
which starves SWDGE descriptor generation (observed as an ~8us delayed
first output). All 8 input slots and 8 output tiles are SBUF-resident,
so no DMA ever waits on buffer recycling.

Sharding: pure data parallel over the batch dim across 8 NeuronCores.
"""

import numpy as np

N_CORES = 8
B, K = 8192, 2048
N_OUT = 2
SHARD_B = B // N_CORES  # 1024 codewords per core
P = 128                 # SBUF partitions

_compiled = {}


def _build_nc():
    import concourse.bass as bass  # noqa: F401
    import concourse.tile as tile
    from concourse import bacc, mybir

    nc = bacc.Bacc(
        "TRN2",
        target_bir_lowering=False,
        debug=False,
        enable_asserts=False,
    )
    x = nc.dram_tensor("x", [SHARD_B, K], mybir.dt.float32, kind="ExternalInput").ap()
    y = nc.dram_tensor(
        "y", [SHARD_B, N_OUT * K], mybir.dt.float32, kind="ExternalOutput"
    ).ap()

    n_groups = SHARD_B // P  # 8 row-groups of 128

    with tile.TileContext(nc) as tc:
        with (
            tc.tile_pool(name="xin", bufs=1) as in_pool,
            tc.tile_pool(name="out", bufs=1) as out_pool,
        ):
            # Persistent input slots with 2 leading zero columns so the
            # shifted views u[t-1], u[t-2] fall out of plain column offsets.
            in_slots = [
                in_pool.tile(
                    [P, K + 2], mybir.dt.float32, tag=f"xin{j}", name=f"xin{j}"
                )
                for j in range(n_groups)
            ]
            out_slots = [
                out_pool.tile(
                    [P, N_OUT * K], mybir.dt.float32, tag=f"out{j}", name=f"out{j}"
                )
                for j in range(n_groups)
            ]
            for j in range(n_groups):
                nc.vector.memset(in_slots[j][:, 0:2], 0.0)

            # All input DMAs upfront, alternating between the two HWDGE
            # rings (SP and ACT) so read descriptors stream from two
            # independent queues.
            for g in range(n_groups):
                rows = slice(g * P, (g + 1) * P)
                eng = nc.sync if g % 2 == 0 else nc.scalar
                eng.dma_start(in_slots[g][:, 2 : 2 + K], x[rows, :])

            for g in range(n_groups):
                xin = in_slots[g]
                rows = slice(g * P, (g + 1) * P)
                a = xin[:, 2 : 2 + K].bitcast(mybir.dt.uint32)  # u[t]
                b = xin[:, 1 : 1 + K].bitcast(mybir.dt.uint32)  # u[t-1]
                c = xin[:, 0:K].bitcast(mybir.dt.uint32)        # u[t-2]

                out = out_slots[g]
                even = out[:, 0 : N_OUT * K : 2].bitcast(mybir.dt.uint32)
                odd = out[:, 1 : N_OUT * K : 2].bitcast(mybir.dt.uint32)

                # out0 = a ^ c ; out1 = out0 ^ b  (bitwise on f32 payloads)
                nc.vector.tensor_tensor(even, a, c, mybir.AluOpType.bitwise_xor)
                nc.vector.tensor_tensor(odd, even, b, mybir.AluOpType.bitwise_xor)

                # Output DMAs also on the HWDGE rings (GpSimd stays fully
                # idle: continuous DVE 2-port ops would starve SWDGE
                # descriptor generation via the shared SBUF port pair).
                oeng = nc.sync if g % 2 == 0 else nc.scalar
                oeng.dma_start(y[rows, :], out[:])

    nc.compile()
    return nc


def _get_nc():
    if "nc" not in _compiled:
        _compiled["nc"] = _build_nc()
    return _compiled["nc"]


def kernel(**inputs) -> np.ndarray:
    from concourse.bass_utils import run_bass_kernel_spmd

    x_full = np.ascontiguousarray(np.asarray(inputs["inputs"], dtype=np.float32))
    assert x_full.shape == (B, K), x_full.shape

    nc = _get_nc()
    in_maps = [
        {"x": x_full[i * SHARD_B : (i + 1) * SHARD_B]} for i in range(N_CORES)
    ]
    res = run_bass_kernel_spmd(nc, in_maps, core_ids=list(range(N_CORES)))
    out = np.concatenate([r["y"] for r in res.results], axis=0)
    return np.ascontiguousarray(out, dtype=np.float32)
